# revision 13
# baseline (speedup 1.0000x reference)
"""GNN SAGE encoder (3x SAGEConv mean-aggr + ReLU, sum-pool per graph) on 8 trn2 cores.

Sharding: nodes are permuted (in-degree LPT-balanced) into 128-node tiles,
tiles dealt to 8 cores.  Edges live with the core owning their dst node.
Each layer, every core gathers source rows from a replicated full node table
(bf16, indirect DMA), aggregates them via weighted one-hot matmuls into PSUM
(fp32), applies the dense SAGE update + ReLU, and AllGathers its shard of the
new node table.  h_T (feature-major own shard) stays resident in SBUF across
layers.  The [G, D] pooled output is computed as per-core partials via a
batch one-hot matmul and summed on the host.
"""

import os
import sys

sys.path.insert(0, "/opt/trn_rl_repo")

import numpy as np
import ml_dtypes

NCORES = 8
P = 128
G = 512  # graphs per batch (fixed by problem spec)
L = 3

_PROG_CACHE = {}
LAST_RESULTS = None  # for test harness introspection


_ABLATE = frozenset()  # timing-ablation switches (bench-only, results invalid)


def _build_program(T, C, NFULL, D, reps=1, ablate=None):
    """T: tiles per core, C: 128-edge chunks per tile, NFULL: node-table rows."""
    ablate = _ABLATE if ablate is None else frozenset(ablate)
    import concourse.bacc as bacc
    import concourse.bass as bass
    import concourse.mybir as mybir
    import concourse.tile as tile
    from concourse.masks import make_identity

    fp32 = mybir.dt.float32
    bf16 = mybir.dt.bfloat16
    i32 = mybir.dt.int32
    Relu = mybir.ActivationFunctionType.Relu

    NROWS = T * P
    NCH = T * C
    assert T % 2 == 0

    nc = bacc.Bacc("TRN2", target_bir_lowering=False, debug=False, num_devices=NCORES)

    xe = nc.dram_tensor("xe", [(T // 2) * P, 2 * C * D], bf16, kind="ExternalInput")
    xt = nc.dram_tensor("xt", [P, NROWS], bf16, kind="ExternalInput")
    gidx = nc.dram_tensor("gidx", [P, NCH], i32, kind="ExternalInput")
    dstc = nc.dram_tensor("dstc", [P, NCH], fp32, kind="ExternalInput")
    wc = nc.dram_tensor("wc", [P, NCH], fp32, kind="ExternalInput")
    bkt = nc.dram_tensor("bkt", [P, T], fp32, kind="ExternalInput")
    Wio = []
    for l in range(L):
        Wio.append(
            (
                nc.dram_tensor(f"Wl{l}", [D, D], bf16, kind="ExternalInput"),
                nc.dram_tensor(f"Wr{l}", [D, D], bf16, kind="ExternalInput"),
                nc.dram_tensor(f"bl{l}", [D, 1], fp32, kind="ExternalInput"),
            )
        )
    pool_out = nc.dram_tensor("pool", [D, G], fp32, kind="ExternalOutput")

    hNM = [None]  # layer 0 reads the host-pre-gathered xe instead
    hsh = []
    for l in range(1, L):
        hNM.append(
            nc.dram_tensor(
                f"hnm{l}",
                [NFULL, D],
                bf16,
                addr_space="Local" if os.environ.get("GNN_HNM_LOCAL") else "Shared",
            )
        )
        hsh.append(nc.dram_tensor(f"hsh{l}", [NROWS, D], bf16))

    rg = [list(range(NCORES))]

    with tile.TileContext(nc) as tc:
        with (
            tc.tile_pool(name="const", bufs=1) as cpool,
            tc.tile_pool(name="gath", bufs=12) as gpool,
            tc.tile_pool(name="work", bufs=4) as wpool,
            tc.tile_pool(name="pm", bufs=2, space="PSUM") as psum_m,
            tc.tile_pool(name="pz", bufs=2, space="PSUM") as psum_z,
            tc.tile_pool(name="pt", bufs=2, space="PSUM") as psum_t,
            tc.tile_pool(name="pp", bufs=1, space="PSUM") as psum_p,
        ):
            # --- stationary data ---
            gidx_s = cpool.tile([P, NCH], i32)
            nc.sync.dma_start(gidx_s[:], gidx[:])
            dstc_s = cpool.tile([P, NCH], fp32)
            nc.sync.dma_start(dstc_s[:], dstc[:])
            wc_s = cpool.tile([P, NCH], fp32)
            nc.sync.dma_start(wc_s[:], wc[:])
            bkt_s = cpool.tile([P, T], fp32)
            nc.sync.dma_start(bkt_s[:], bkt[:])

            M = max(G, P)
            iota_i = cpool.tile([P, M], i32)
            nc.gpsimd.iota(iota_i[:], pattern=[[1, M]], base=0, channel_multiplier=0)
            iotaG = cpool.tile([P, M], fp32)
            nc.vector.tensor_copy(iotaG[:], iota_i[:])
            iotaPb = cpool.tile([P, P], bf16)
            nc.vector.tensor_copy(iotaPb[:], iota_i[:, 0:P])

            ident = cpool.tile([P, P], bf16)
            make_identity(nc, ident[:])

            W_s = []
            for l in range(L):
                Wl_s = cpool.tile([P, P], bf16, tag=f"wl{l}")
                nc.sync.dma_start(Wl_s[:], Wio[l][0][:])
                Wr_s = cpool.tile([P, P], bf16, tag=f"wr{l}")
                nc.sync.dma_start(Wr_s[:], Wio[l][1][:])
                bl_s = cpool.tile([P, 1], fp32, tag=f"bl{l}")
                nc.sync.dma_start(bl_s[:], Wio[l][2][:])
                W_s.append((Wl_s, Wr_s, bl_s))

            # h_T shard, feature-major, resident in SBUF; buf l is read by
            # layer l, buf l+1 written (3 bufs so reps>1 stays idempotent).
            hTb = [
                cpool.tile([P, NROWS], bf16, tag=f"hT{i}", name=f"hT{i}")
                for i in range(L)
            ]
            nc.sync.dma_start(hTb[0][:], xt[:])

            ppool = psum_p.tile([P, G], fp32)

            for rep in range(reps):
                for l in range(L):
                    Wl_s, Wr_s, bl_s = W_s[l]
                    for tp in range(T // 2):
                        t0 = 2 * tp
                        # mean_T[f, n] for the tile pair, fp32 accumulate
                        pm = psum_m.tile([P, 2 * P], fp32, tag="pm")
                        Hgp = None
                        if l == 0 and "gather" not in ablate:
                            # layer 0: stream the pre-gathered edge rows
                            Hgp = gpool.tile([P, 2 * C * P], bf16, tag="Hgp")
                            nc.sync.dma_start(
                                Hgp[:], xe[tp * P : (tp + 1) * P, :]
                            )
                        for dt_ in range(2):
                            t = t0 + dt_
                            for c in range(C):
                                j = t * C + c
                                if l == 0:
                                    if Hgp is not None:
                                        Hg = Hgp[:, (dt_ * C + c) * D : (dt_ * C + c + 1) * D]
                                    else:
                                        Hg0 = gpool.tile([P, D], bf16, tag="Hg")
                                        nc.vector.memset(Hg0[:], 0.0)
                                        Hg = Hg0[:]
                                    S = gpool.tile([P, P], bf16, tag="S")
                                    if "onehot" in ablate:
                                        nc.scalar.copy(S[:], iotaPb[:])
                                    else:
                                        nc.vector.tensor_scalar(
                                            out=S[:],
                                            in0=iotaPb[:],
                                            scalar1=dstc_s[:, j : j + 1],
                                            scalar2=wc_s[:, j : j + 1],
                                            op0=mybir.AluOpType.is_equal,
                                            op1=mybir.AluOpType.mult,
                                        )
                                    if "aggmm" in ablate:
                                        if c == 0:
                                            nc.vector.memset(pm[:, dt_ * P : (dt_ + 1) * P], 0.0)
                                    else:
                                        nc.tensor.matmul(
                                            pm[:, dt_ * P : (dt_ + 1) * P],
                                            lhsT=Hg,
                                            rhs=S[:],
                                            start=(c == 0),
                                            stop=(c == C - 1),
                                        )
                                    continue
                                Hg = gpool.tile([P, D], bf16, tag="Hg")
                                if "gathseq" in ablate:
                                    nc.gpsimd.dma_start(
                                        Hg[:], hNM[l][(j % T) * P : (j % T + 1) * P, :]
                                    )
                                elif "gather" in ablate:
                                    nc.vector.memset(Hg[:], 0.0)
                                else:
                                    nc.gpsimd.indirect_dma_start(
                                        out=Hg[:],
                                        out_offset=None,
                                        in_=hNM[l][:],
                                        in_offset=bass.IndirectOffsetOnAxis(
                                            ap=gidx_s[:, j : j + 1], axis=0
                                        ),
                                    )
                                S = gpool.tile([P, P], bf16, tag="S")
                                if "onehot" in ablate:
                                    nc.scalar.copy(S[:], iotaPb[:])
                                if "onehot" not in ablate:
                                    nc.vector.tensor_scalar(
                                        out=S[:],
                                        in0=iotaPb[:],
                                        scalar1=dstc_s[:, j : j + 1],
                                        scalar2=wc_s[:, j : j + 1],
                                        op0=mybir.AluOpType.is_equal,
                                        op1=mybir.AluOpType.mult,
                                    )
                                if "aggmm" in ablate:
                                    if c == 0:
                                        nc.vector.memset(pm[:, dt_ * P : (dt_ + 1) * P], 0.0)
                                else:
                                    nc.tensor.matmul(
                                        pm[:, dt_ * P : (dt_ + 1) * P],
                                        lhsT=Hg[:],
                                        rhs=S[:],
                                        start=(c == 0),
                                        stop=(c == C - 1),
                                    )
                        meanT = wpool.tile([P, 2 * P], bf16, tag="meanT")
                        nc.scalar.copy(meanT[:], pm[:])
                        pz = psum_z.tile([P, 2 * P], fp32, tag="pz")
                        if "dense" in ablate:
                            nc.vector.memset(pz[:], 0.0)
                        if "dense" not in ablate:
                            nc.tensor.matmul(
                                pz[:], lhsT=Wl_s[:], rhs=meanT[:], start=True, stop=False
                            )
                            nc.tensor.matmul(
                                pz[:],
                                lhsT=Wr_s[:],
                                rhs=hTb[l][:, t0 * P : (t0 + 2) * P],
                                start=False,
                                stop=True,
                            )
                        if l < L - 1:
                            hnT = hTb[l + 1][:, t0 * P : (t0 + 2) * P]
                        else:
                            hnT_t = wpool.tile([P, 2 * P], bf16, tag="hnT")
                            hnT = hnT_t[:]
                        nc.scalar.activation(hnT, pz[:], Relu, bias=bl_s[:, 0:1])
                        for dt_ in range(2):
                            t = t0 + dt_
                            ptr = psum_t.tile([P, P], bf16, tag="ptr")
                            if "transp" in ablate:
                                nc.vector.memset(ptr[:], 0.0)
                            if "transp" not in ablate:
                                nc.tensor.transpose(
                                    ptr[:], hnT[:, dt_ * P : (dt_ + 1) * P], ident[:]
                                )
                            hnm = wpool.tile([P, P], bf16, tag="hnm")
                            if "copy" in ablate:
                                nc.vector.memset(hnm[:], 0.0)
                            if "copy" not in ablate:
                                nc.any.tensor_copy(hnm[:], ptr[:])
                            if l < L - 1:
                                if "store" not in ablate:
                                    nc.sync.dma_start(
                                        hsh[l][t * P : (t + 1) * P, :], hnm[:]
                                    )
                            else:
                                B = wpool.tile([P, G], bf16, tag="B")
                                nc.vector.tensor_scalar(
                                    out=B[:],
                                    in0=iotaG[:, 0:G],
                                    scalar1=bkt_s[:, t : t + 1],
                                    scalar2=None,
                                    op0=mybir.AluOpType.is_equal,
                                )
                                nc.tensor.matmul(
                                    ppool[:],
                                    lhsT=hnm[:],
                                    rhs=B[:],
                                    start=(t == 0),
                                    stop=(t == T - 1),
                                )
                    if l < L - 1 and "ag" not in ablate:
                        nc.gpsimd.collective_compute(
                            "AllGather",
                            mybir.AluOpType.bypass,
                            replica_groups=rg,
                            ins=[hsh[l][:]],
                            outs=[hNM[l + 1][:]],
                        )

            poolsb = wpool.tile([P, G], fp32, tag="poolsb")
            nc.vector.tensor_copy(poolsb[:], ppool[:])
            nc.sync.dma_start(pool_out[:], poolsb[:])

    nc.compile()
    return nc


def _balance_tiles(deg, NFULL):
    """LPT: place nodes (desc by in-degree) into NFULL/P tiles of 128 slots,
    minimizing max tile in-degree.  Returns new_pos (padded-node-id -> slot)."""
    import heapq

    n_tiles = NFULL // P
    order = np.argsort(-deg, kind="stable")
    new_pos = np.empty(NFULL, dtype=np.int64)
    heap = [(0, t) for t in range(n_tiles)]
    fill = np.zeros(n_tiles, dtype=np.int32)
    # place positive-degree nodes by LPT
    degs = deg[order]
    npos = int(np.searchsorted(-degs, 0))  # count of deg>0 entries
    for i in range(npos):
        load, t = heapq.heappop(heap)
        new_pos[order[i]] = t * P + fill[t]
        fill[t] += 1
        if fill[t] < P:
            heapq.heappush(heap, (load + int(degs[i]), t))
    # zero-degree + dummy nodes fill remaining slots
    rem = []
    for t in range(n_tiles):
        for s in range(fill[t], P):
            rem.append(t * P + s)
    new_pos[order[npos:]] = np.array(rem, dtype=np.int64)
    return new_pos


def _preprocess(x, edge_index, batch):
    N, F = x.shape
    src, dst = np.asarray(edge_index[0]), np.asarray(edge_index[1])
    E = src.shape[0]

    n_tiles = ((N + P - 1) // P + NCORES - 1) // NCORES * NCORES
    T = n_tiles // NCORES
    if T % 2:
        n_tiles += NCORES
        T += 1
    NFULL = n_tiles * P

    deg = np.bincount(dst, minlength=N).astype(np.int64)
    inv_deg = (1.0 / np.maximum(deg, 1)).astype(np.float32)

    deg_pad = np.zeros(NFULL, dtype=np.int64)
    deg_pad[:N] = deg
    new_pos = _balance_tiles(deg_pad, NFULL)

    tile_load = np.bincount(new_pos[dst] // P, minlength=n_tiles)
    C = max(1, int(-(-tile_load.max() // P)))

    dst_new = new_pos[dst]
    src_new = new_pos[src]
    eo = np.argsort(dst_new, kind="stable")
    dst_s, src_s = dst_new[eo], src_new[eo]
    t_e = dst_s // P
    counts = np.bincount(t_e, minlength=n_tiles)
    starts = np.concatenate([[0], np.cumsum(counts)[:-1]])
    pos_in_tile = np.arange(E) - starts[t_e]

    CP = C * P
    gsrc = np.zeros((n_tiles, CP), dtype=np.int32)
    dslot = np.zeros((n_tiles, CP), dtype=np.float32)
    wpad = np.zeros((n_tiles, CP), dtype=np.float32)
    gsrc[t_e, pos_in_tile] = src_s.astype(np.int32)
    dslot[t_e, pos_in_tile] = (dst_s % P).astype(np.float32)
    wpad[t_e, pos_in_tile] = inv_deg[dst[eo]]

    x_perm = np.zeros((NFULL, F), dtype=ml_dtypes.bfloat16)
    x_perm[new_pos[:N]] = np.asarray(x, dtype=np.float32).astype(ml_dtypes.bfloat16)

    batch_local = np.full(NFULL, -1.0, dtype=np.float32)
    batch_local[new_pos[:N]] = np.asarray(batch, dtype=np.float32)

    per_core = []
    for k in range(NCORES):
        tl = slice(k * T, (k + 1) * T)
        rows = slice(k * T * P, (k + 1) * T * P)

        def cols(a):
            return np.ascontiguousarray(
                a[tl].reshape(T, C, P).transpose(2, 0, 1).reshape(P, T * C)
            )

        per_core.append(
            {
                "gidx": cols(gsrc),
                "dstc": cols(dslot),
                "wc": cols(wpad),
                # layer-0 edge rows pre-gathered on host, laid out so that
                # tile-pair tp loads a contiguous [P, 2*C*D] block
                "xe": np.ascontiguousarray(
                    x_perm[gsrc[tl].reshape(-1)]
                    .reshape(T // 2, 2 * C, P, F)
                    .transpose(0, 2, 1, 3)
                    .reshape((T // 2) * P, 2 * C * F)
                ),
                "xt": np.ascontiguousarray(x_perm[rows].T),
                "bkt": np.ascontiguousarray(batch_local[rows].reshape(T, P).T),
            }
        )
    return x_perm, per_core, T, C, NFULL


def _make_in_maps(inputs, x_perm, per_core, D):
    in_maps = []
    for k in range(NCORES):
        m = {}
        m.update(per_core[k])
        for l in range(L):
            m[f"Wl{l}"] = np.asarray(inputs[f"Wl{l}"], np.float32).astype(
                ml_dtypes.bfloat16
            )
            m[f"Wr{l}"] = np.asarray(inputs[f"Wr{l}"], np.float32).astype(
                ml_dtypes.bfloat16
            )
            m[f"bl{l}"] = np.ascontiguousarray(
                np.asarray(inputs[f"bl{l}"], dtype=np.float32).reshape(D, 1)
            )
        in_maps.append(m)
    return in_maps


def kernel(**inputs):
    global LAST_RESULTS
    from concourse.bass_utils import run_bass_kernel_spmd

    x = np.asarray(inputs["x"], dtype=np.float32)
    edge_index = np.asarray(inputs["edge_index"])
    batch = np.asarray(inputs["batch"])
    D = inputs["Wl0"].shape[1]

    x_perm, per_core, T, C, NFULL = _preprocess(x, edge_index, batch)

    key = (T, C, NFULL, D)
    if key not in _PROG_CACHE:
        _PROG_CACHE[key] = _build_program(T, C, NFULL, D)
    nc = _PROG_CACHE[key]

    in_maps = _make_in_maps(inputs, x_perm, per_core, D)
    res = run_bass_kernel_spmd(nc, in_maps, list(range(NCORES)))
    LAST_RESULTS = res

    pool = np.zeros((D, G), dtype=np.float64)
    for k in range(NCORES):
        pool += res.results[k]["pool"].astype(np.float64)
    return np.ascontiguousarray(pool.T).astype(np.float32)
